# revision 15
# baseline (speedup 1.0000x reference)
"""TRN2 Bass kernel for nn_DivTree (moe_routing): per-agent 2-layer MLP.

Math (per batch row b, agent a, with r = routing[a]):
    x0   = concat(x_in[b, a], onehot(a))                  # [H + A]
    h    = relu(x0 @ W1[r] + b1[r])                       # [H]
    out  = h @ W2[r] + b2[r]                              # [NACT]

Host-side simplifications baked in before the device kernel runs:
  - The onehot half of x0 @ W1[r] selects row H+a of W1[r]; folded into an
    effective bias: bias1e[a] = b1[r] + W1[r, H+a, :].
  - Expert weights gathered by routing on the host (pure indexing).
  - x and W cast to bf16 (measured end-to-end err ~2.8e-3 rel); fp32 PSUM.
    bf16 matmuls stream 1 row/cycle like fp32r but their 32KB weight loads
    hide fully under the 213ns stream (fp32r's 64KB loads don't), and DMA
    traffic halves. fp8 DoubleRow was measured at 2x MACs/instr wall-equal,
    which loses once the >=2 correction terms needed for accuracy are paid.

Sharding: expert-parallel over agents. 48 agents assigned whole to cores
(6 each); agents 48/49 split into 4 batch-quarters (cores 0-3 / 4-7), so
all 8 cores run an identical program over 25 (agent, batch-512) units.

Device kernel per unit: 16 bf16 matmuls (L1) + 4 bf16 matmuls (L2, flushed
one unit late to keep the PE streaming). Relu+bias+bf16-cast runs on the
Vector engine (tensor_scalar add+max), keeping Scalar free for weight DMA;
xt DMA on Sync, output DMA on GpSimd.
"""

import os
import sys

import numpy as np

sys.path.insert(0, "/opt/trn_rl_repo")

B, A, H, NACT = 2048, 50, 512, 64
N_CORES = 8
BT = 512  # batch tile (rows per work unit)
FULL_PER_CORE = 6  # whole agents per core
N_UNITS = FULL_PER_CORE * 4 + 1  # 25 work units per core
N_AG = FULL_PER_CORE + 1  # weight slots per core (6 full + 1 split)
KC = H // 128  # 4 contraction k-tiles
MC = H // 128  # 4 output-hidden chunks

LAST_RUN_INFO = {}

_CACHE = {}


def _unit_tables():
    """Per-core unit -> (agent, b0) and weight-slot tables."""
    per_core = []
    for c in range(N_CORES):
        full = list(range(c * FULL_PER_CORE, (c + 1) * FULL_PER_CORE))
        split_agent = 48 + (c // 4)
        quarter = c % 4
        units = [(a, j * BT) for a in full for j in range(4)]
        units.append((split_agent, quarter * BT))
        agents = full + [split_agent]
        per_core.append((units, agents))
    return per_core


def _build_nc():
    import concourse.bacc as bacc
    import concourse.mybir as mybir
    import concourse.tile as tile

    F32 = mybir.dt.float32
    BF16 = mybir.dt.bfloat16
    ADD = mybir.AluOpType.add
    MAX = mybir.AluOpType.max
    Relu = mybir.ActivationFunctionType.Relu

    nc = bacc.Bacc(None)
    # x: [unit][partition][ktile][batch]
    xt_d = nc.declare_dram_parameter("xt", [N_UNITS, 128, KC, BT], BF16, isOutput=False)
    # W1: [agent][partition][ktile][m][col]
    w1_d = nc.declare_dram_parameter(
        "w1", [N_AG, 128, KC, MC, 128], BF16, isOutput=False
    )
    w2_d = nc.declare_dram_parameter("w2", [N_AG, 128, KC, NACT], BF16, isOutput=False)
    # bias: cols 0..MC-1 = bias1e (m-chunks), col MC = b2
    bs_d = nc.declare_dram_parameter("bs", [N_AG, 128, MC + 1], F32, isOutput=False)
    out_d = nc.declare_dram_parameter("out", [N_AG, NACT, 4 * BT], F32, isOutput=True)

    with tile.TileContext(nc) as tc:
        with (
            tc.tile_pool(name="xtp", bufs=6) as xtp,
            tc.tile_pool(name="w1p", bufs=3) as w1p,
            tc.tile_pool(name="w2p", bufs=3) as w2p,
            tc.tile_pool(name="bsp", bufs=3) as bsp,
            tc.tile_pool(name="htp", bufs=8) as htp,
            tc.tile_pool(name="obp", bufs=3) as obp,
            tc.tile_pool(name="ps1p", bufs=5, space="PSUM") as ps1p,
            tc.tile_pool(name="ps2p", bufs=3, space="PSUM") as ps2p,
        ):
            w1ts, w2ts, bsts = {}, {}, {}

            def emit_wslab(ai, split_first=False):
                w1t = w1p.tile([128, KC, MC, 128], BF16, tag="w1", name=f"w1_{ai}")
                if split_first:
                    # stream k-by-k so unit 0's k-outer matmuls start early
                    # (k=0 finer still, by m-block, to unblock the very
                    # first matmul on a 32KB transfer)
                    for m in range(MC):
                        nc.scalar.dma_start(out=w1t[:, 0, m], in_=w1_d[ai][:, 0, m])
                    for k in range(1, KC):
                        nc.scalar.dma_start(out=w1t[:, k], in_=w1_d[ai][:, k])
                else:
                    nc.scalar.dma_start(out=w1t, in_=w1_d[ai])
                w2t = w2p.tile([128, KC, NACT], BF16, tag="w2", name=f"w2_{ai}")
                nc.scalar.dma_start(out=w2t, in_=w2_d[ai])
                bst = bsp.tile([128, MC + 1], F32, tag="bs", name=f"bs_{ai}")
                nc.scalar.dma_start(out=bst, in_=bs_d[ai])
                w1ts[ai], w2ts[ai], bsts[ai] = w1t, w2t, bst

            def flush(p):
                hts, ai, u, j = p
                ps2 = ps2p.tile([128, BT], F32, tag="ps2", name=f"ps2_{u}")
                w2t = w2ts[ai]
                for k in range(KC):
                    nc.tensor.matmul(
                        ps2[0:NACT, :],
                        lhsT=w2t[:, k, :],
                        rhs=hts[k],
                        start=(k == 0),
                        stop=(k == KC - 1),
                    )
                ob = obp.tile([NACT, BT], F32, tag="ob", name=f"ob_{u}")
                nc.vector.tensor_scalar_add(
                    out=ob, in0=ps2[0:NACT, :], scalar1=bsts[ai][0:NACT, MC : MC + 1]
                )
                nc.gpsimd.dma_start(out=out_d[ai][:, j * BT : (j + 1) * BT], in_=ob)

            emit_wslab(0, split_first=True)

            # Warm the PE (clock ramp) with dummy bf16 matmuls while the
            # first xt/w1 DMAs stream in.
            warm = htp.tile([128, BT + NACT], BF16, tag="warm", name="warm", bufs=1)
            nc.gpsimd.memset(warm, 0.0)
            wps = ps2p.tile([128, BT], F32, tag="ps2", name="warm_ps")
            NWARM = 4
            for r in range(NWARM):
                nc.tensor.matmul(
                    wps[0:NACT, : BT // 2],
                    lhsT=warm[:, BT : BT + NACT],
                    rhs=warm[:, : BT // 2],
                    start=(r == 0),
                    stop=(r == NWARM - 1),
                )

            pending = None
            for u in range(N_UNITS):
                ai = u // 4 if u < FULL_PER_CORE * 4 else FULL_PER_CORE
                j = (u % 4) if ai != FULL_PER_CORE else 0

                xt = xtp.tile([128, KC, BT], BF16, tag="xt", name=f"xt_{u}")
                if u == 0:
                    for k in range(KC):
                        nc.sync.dma_start(out=xt[:, k], in_=xt_d[u][:, k])
                else:
                    nc.sync.dma_start(out=xt, in_=xt_d[u])
                if u % 4 == 0 and u > 0 and u // 4 + 1 <= FULL_PER_CORE:
                    emit_wslab(u // 4 + 1)  # one-agent prefetch lead
                if u == 1:
                    emit_wslab(1)

                w1t = w1ts[ai]
                ps1s = [
                    ps1p.tile([128, BT], F32, tag="ps1", name=f"ps1_{u}_{m}")
                    for m in range(MC)
                ]
                # Unit 0 runs k-outer so each arriving xt/w1 k-chunk unlocks
                # 4 matmuls; later units run m-outer so each psum completes
                # early and its relu overlaps the next m.
                order = (
                    [(m, k) for k in range(KC) for m in range(MC)]
                    if u == 0
                    else [(m, k) for m in range(MC) for k in range(KC)]
                )
                hts = [None] * MC
                for m, k in order:
                    nc.tensor.matmul(
                        ps1s[m],
                        lhsT=w1t[:, k, m, :],
                        rhs=xt[:, k, :],
                        start=(k == 0),
                        stop=(k == KC - 1),
                    )
                    if k == KC - 1:
                        ht = htp.tile([128, BT], BF16, tag="ht", name=f"ht_{u}_{m}")
                        # alternate relu between ACT and DVE so neither
                        # engine's psum drain gates the PE's psum reuse
                        if m % 2 == 0:
                            nc.scalar.activation(
                                out=ht,
                                in_=ps1s[m],
                                func=Relu,
                                bias=bsts[ai][:, m : m + 1],
                            )
                        else:
                            nc.vector.tensor_scalar(
                                out=ht,
                                in0=ps1s[m],
                                scalar1=bsts[ai][:, m : m + 1],
                                scalar2=0.0,
                                op0=ADD,
                                op1=MAX,
                            )
                        hts[m] = ht

                if pending is not None:
                    flush(pending)
                pending = (hts, ai, u, j)
            flush(pending)

    nc.finalize()
    return nc


def _prep_inputs(x_in, W1, b1, W2, b2, routing):
    """Host-side: routing gather, onehot fold, bf16 cast, per-core tiling."""
    import ml_dtypes

    BF = ml_dtypes.bfloat16

    x_in = np.ascontiguousarray(x_in, dtype=np.float32)
    W1 = np.asarray(W1, dtype=np.float32)
    b1 = np.asarray(b1, dtype=np.float32)
    W2 = np.asarray(W2, dtype=np.float32)
    b2 = np.asarray(b2, dtype=np.float32)
    routing = np.asarray(routing)

    W1r = W1[routing]  # [A, H+A, H]
    W2r = W2[routing]  # [A, H, NACT]
    bias1e = b1[routing] + W1r[np.arange(A), H + np.arange(A), :]  # [A, H]
    b2e = b2[routing]  # [A, NACT]
    W1h = W1r[:, :H, :]  # [A, H, H]

    x_bf = x_in.astype(BF)  # [B, A, H]

    # [A, H, H] -> [A, 128, KC, MC, 128]: [a, kp, k, m, mc] = W[a, k*128+kp, m*128+mc]
    w1_all = np.ascontiguousarray(
        W1h.astype(BF).reshape(A, KC, 128, MC, 128).transpose(0, 2, 1, 3, 4)
    )
    w2_all = np.ascontiguousarray(
        W2r.astype(BF).reshape(A, KC, 128, NACT).transpose(0, 2, 1, 3)
    )  # [A, 128, KC, NACT]

    bs_all = np.zeros((A, 128, MC + 1), dtype=np.float32)
    bs_all[:, :, :MC] = bias1e.reshape(A, MC, 128).transpose(0, 2, 1)
    bs_all[:, :NACT, MC] = b2e

    per_core = _unit_tables()
    in_maps = []
    for c in range(N_CORES):
        units, agents = per_core[c]
        xt = np.empty((N_UNITS, 128, KC, BT), dtype=BF)
        for u, (a, b0) in enumerate(units):
            # [kp, k, b] = x[b0+b, a, k*128+kp]
            xt[u] = x_bf[b0 : b0 + BT, a, :].T.reshape(KC, 128, BT).transpose(1, 0, 2)
        in_maps.append(
            {
                "xt": xt,
                "w1": np.ascontiguousarray(w1_all[agents]),
                "w2": w2_all[agents],
                "bs": bs_all[agents],
            }
        )
    return in_maps, per_core


def _install_ntff_hook():
    import types

    try:
        from antenv.axon_hooks import get_axon_ntff_profile_hook  # noqa: F401

        return
    except ImportError:
        pass
    try:
        import antenv
        from trn_agent_boot.trn_boot import _ntff_profile_via_ctypes

        hook = _ntff_profile_via_ctypes("/opt/axon/libaxon_pjrt.so")
        mod = types.ModuleType("antenv.axon_hooks")
        mod.get_axon_ntff_profile_hook = lambda: hook
        mod.set_axon_ntff_profile_hook = lambda h: None
        sys.modules["antenv.axon_hooks"] = mod
        antenv.axon_hooks = mod
    except Exception:
        pass


def kernel(x_in, W1, b1, W2, b2, routing):
    from concourse.bass_utils import run_bass_kernel_spmd

    trace = bool(os.environ.get("TRN_KERNEL_TRACE"))
    if trace:
        _install_ntff_hook()

    if "nc" not in _CACHE:
        _CACHE["nc"] = _build_nc()
    nc = _CACHE["nc"]

    in_maps, per_core = _prep_inputs(x_in, W1, b1, W2, b2, routing)

    kwargs = {}
    if trace:
        kwargs = dict(trace=True, tmpdir=os.environ.get("TRN_KERNEL_TRACE_DIR") or None)
    res = run_bass_kernel_spmd(nc, in_maps, core_ids=list(range(N_CORES)), **kwargs)

    LAST_RUN_INFO.clear()
    LAST_RUN_INFO["exec_time_ns"] = res.exec_time_ns
    LAST_RUN_INFO["results"] = res

    out_full = np.empty((B, A, NACT), dtype=np.float32)
    for c in range(N_CORES):
        units, agents = per_core[c]
        oc = res.results[c]["out"]  # [N_AG, NACT, 4*BT]
        for ai, a in enumerate(agents):
            if ai == FULL_PER_CORE:
                b0 = units[-1][1]
                out_full[b0 : b0 + BT, a, :] = oc[ai, :, :BT].T
            else:
                out_full[:, a, :] = oc[ai].T
    return out_full


# revision 16
# speedup vs baseline: 1.0353x; 1.0353x over previous
"""TRN2 Bass kernel for nn_DivTree (moe_routing): per-agent 2-layer MLP.

Math (per batch row b, agent a, with r = routing[a]):
    x0   = concat(x_in[b, a], onehot(a))                  # [H + A]
    h    = relu(x0 @ W1[r] + b1[r])                       # [H]
    out  = h @ W2[r] + b2[r]                              # [NACT]

Host-side simplifications baked in before the device kernel runs:
  - The onehot half of x0 @ W1[r] selects row H+a of W1[r]; folded into an
    effective bias: bias1e[a] = b1[r] + W1[r, H+a, :].
  - Expert weights gathered by routing on the host (pure indexing).
  - x and W cast to bf16 (measured end-to-end err ~2.8e-3 rel); fp32 PSUM.
    bf16 matmuls stream 1 row/cycle like fp32r but their 32KB weight loads
    hide fully under the 213ns stream (fp32r's 64KB loads don't), and DMA
    traffic halves. fp8 DoubleRow was measured at 2x MACs/instr wall-equal,
    which loses once the >=2 correction terms needed for accuracy are paid.

Sharding: expert-parallel over agents. 48 agents assigned whole to cores
(6 each); agents 48/49 split into 4 batch-quarters (cores 0-3 / 4-7), so
all 8 cores run an identical program over 25 (agent, batch-512) units.

Device kernel per unit: 16 bf16 matmuls (L1) + 4 bf16 matmuls (L2, flushed
one unit late to keep the PE streaming). Relu+bias+bf16-cast runs on the
Vector engine (tensor_scalar add+max), keeping Scalar free for weight DMA;
xt DMA on Sync, output DMA on GpSimd.
"""

import os
import sys

import numpy as np

sys.path.insert(0, "/opt/trn_rl_repo")

B, A, H, NACT = 2048, 50, 512, 64
N_CORES = 8
BT = 512  # batch tile (rows per work unit)
FULL_PER_CORE = 6  # whole agents per core
N_UNITS = FULL_PER_CORE * 4 + 1  # 25 work units per core
N_AG = FULL_PER_CORE + 1  # weight slots per core (6 full + 1 split)
KC = H // 128  # 4 contraction k-tiles
MC = H // 128  # 4 output-hidden chunks

LAST_RUN_INFO = {}

_CACHE = {}


def _unit_tables():
    """Per-core unit -> (agent, b0) and weight-slot tables."""
    per_core = []
    for c in range(N_CORES):
        full = list(range(c * FULL_PER_CORE, (c + 1) * FULL_PER_CORE))
        split_agent = 48 + (c // 4)
        quarter = c % 4
        units = [(a, j * BT) for a in full for j in range(4)]
        units.append((split_agent, quarter * BT))
        agents = full + [split_agent]
        per_core.append((units, agents))
    return per_core


def _build_nc():
    import concourse.bacc as bacc
    import concourse.mybir as mybir
    import concourse.tile as tile

    F32 = mybir.dt.float32
    BF16 = mybir.dt.bfloat16
    ADD = mybir.AluOpType.add
    MAX = mybir.AluOpType.max
    Relu = mybir.ActivationFunctionType.Relu

    nc = bacc.Bacc(None)
    # x: [unit][partition][ktile][batch]
    xt_d = nc.declare_dram_parameter("xt", [N_UNITS, 128, KC, BT], BF16, isOutput=False)
    # W1: [agent][partition][ktile][m][col]
    w1_d = nc.declare_dram_parameter(
        "w1", [N_AG, 128, KC, MC, 128], BF16, isOutput=False
    )
    w2_d = nc.declare_dram_parameter("w2", [N_AG, 128, KC, NACT], BF16, isOutput=False)
    # bias: cols 0..MC-1 = bias1e (m-chunks), col MC = b2
    bs_d = nc.declare_dram_parameter("bs", [N_AG, 128, MC + 1], F32, isOutput=False)
    out_d = nc.declare_dram_parameter("out", [N_AG, NACT, 4 * BT], F32, isOutput=True)

    with tile.TileContext(nc) as tc:
        with (
            tc.tile_pool(name="xtp", bufs=6) as xtp,
            tc.tile_pool(name="w1p", bufs=3) as w1p,
            tc.tile_pool(name="w2p", bufs=3) as w2p,
            tc.tile_pool(name="bsp", bufs=3) as bsp,
            tc.tile_pool(name="htp", bufs=8) as htp,
            tc.tile_pool(name="obp", bufs=3) as obp,
            tc.tile_pool(name="ps1p", bufs=6, space="PSUM") as ps1p,
            tc.tile_pool(name="ps2p", bufs=2, space="PSUM") as ps2p,
        ):
            w1ts, w2ts, bsts = {}, {}, {}

            def emit_wslab(ai, split_first=False):
                w1t = w1p.tile([128, KC, MC, 128], BF16, tag="w1", name=f"w1_{ai}")
                if split_first:
                    # stream k-by-k so unit 0's k-outer matmuls start early
                    for k in range(KC):
                        nc.scalar.dma_start(out=w1t[:, k], in_=w1_d[ai][:, k])
                else:
                    nc.scalar.dma_start(out=w1t, in_=w1_d[ai])
                w2t = w2p.tile([128, KC, NACT], BF16, tag="w2", name=f"w2_{ai}")
                nc.scalar.dma_start(out=w2t, in_=w2_d[ai])
                bst = bsp.tile([128, MC + 1], F32, tag="bs", name=f"bs_{ai}")
                nc.scalar.dma_start(out=bst, in_=bs_d[ai])
                w1ts[ai], w2ts[ai], bsts[ai] = w1t, w2t, bst

            def flush(p):
                hts, ai, u, j = p
                ps2 = ps2p.tile([128, BT], F32, tag="ps2", name=f"ps2_{u}")
                w2t = w2ts[ai]
                for k in range(KC):
                    nc.tensor.matmul(
                        ps2[0:NACT, :],
                        lhsT=w2t[:, k, :],
                        rhs=hts[k],
                        start=(k == 0),
                        stop=(k == KC - 1),
                    )
                ob = obp.tile([NACT, BT], F32, tag="ob", name=f"ob_{u}")
                nc.vector.tensor_scalar_add(
                    out=ob, in0=ps2[0:NACT, :], scalar1=bsts[ai][0:NACT, MC : MC + 1]
                )
                nc.gpsimd.dma_start(out=out_d[ai][:, j * BT : (j + 1) * BT], in_=ob)

            emit_wslab(0, split_first=True)

            # Warm the PE (clock ramp) with dummy bf16 matmuls while the
            # first xt/w1 DMAs stream in.
            warm = htp.tile([128, BT + NACT], BF16, tag="warm", name="warm", bufs=1)
            nc.gpsimd.memset(warm, 0.0)
            wps = ps2p.tile([128, BT], F32, tag="ps2", name="warm_ps")
            NWARM = 4
            for r in range(NWARM):
                nc.tensor.matmul(
                    wps[0:NACT, : BT // 2],
                    lhsT=warm[:, BT : BT + NACT],
                    rhs=warm[:, : BT // 2],
                    start=(r == 0),
                    stop=(r == NWARM - 1),
                )

            pending = None
            for u in range(N_UNITS):
                ai = u // 4 if u < FULL_PER_CORE * 4 else FULL_PER_CORE
                j = (u % 4) if ai != FULL_PER_CORE else 0

                xt = xtp.tile([128, KC, BT], BF16, tag="xt", name=f"xt_{u}")
                if u == 0:
                    for k in range(KC):
                        nc.sync.dma_start(out=xt[:, k], in_=xt_d[u][:, k])
                else:
                    nc.sync.dma_start(out=xt, in_=xt_d[u])
                if u % 4 == 0 and u > 0 and u // 4 + 1 <= FULL_PER_CORE:
                    emit_wslab(u // 4 + 1)  # one-agent prefetch lead
                if u == 1:
                    emit_wslab(1)

                w1t = w1ts[ai]
                ps1s = [
                    ps1p.tile([128, BT], F32, tag="ps1", name=f"ps1_{u}_{m}")
                    for m in range(MC)
                ]
                # Unit 0 runs k-outer so each arriving xt/w1 k-chunk unlocks
                # 4 matmuls; later units run m-outer so each psum completes
                # early and its relu overlaps the next m.
                order = (
                    [(m, k) for k in range(KC) for m in range(MC)]
                    if u == 0
                    else [(m, k) for m in range(MC) for k in range(KC)]
                )
                hts = [None] * MC
                for m, k in order:
                    nc.tensor.matmul(
                        ps1s[m],
                        lhsT=w1t[:, k, m, :],
                        rhs=xt[:, k, :],
                        start=(k == 0),
                        stop=(k == KC - 1),
                    )
                    if k == KC - 1:
                        ht = htp.tile([128, BT], BF16, tag="ht", name=f"ht_{u}_{m}")
                        # alternate relu between ACT and DVE so neither
                        # engine's psum drain gates the PE's psum reuse
                        if m % 2 == 0:
                            nc.scalar.activation(
                                out=ht,
                                in_=ps1s[m],
                                func=Relu,
                                bias=bsts[ai][:, m : m + 1],
                            )
                        else:
                            nc.vector.tensor_scalar(
                                out=ht,
                                in0=ps1s[m],
                                scalar1=bsts[ai][:, m : m + 1],
                                scalar2=0.0,
                                op0=ADD,
                                op1=MAX,
                            )
                        hts[m] = ht

                if pending is not None:
                    flush(pending)
                pending = (hts, ai, u, j)
            flush(pending)

    nc.finalize()
    return nc


def _prep_inputs(x_in, W1, b1, W2, b2, routing):
    """Host-side: routing gather, onehot fold, bf16 cast, per-core tiling."""
    import ml_dtypes

    BF = ml_dtypes.bfloat16

    x_in = np.ascontiguousarray(x_in, dtype=np.float32)
    W1 = np.asarray(W1, dtype=np.float32)
    b1 = np.asarray(b1, dtype=np.float32)
    W2 = np.asarray(W2, dtype=np.float32)
    b2 = np.asarray(b2, dtype=np.float32)
    routing = np.asarray(routing)

    W1r = W1[routing]  # [A, H+A, H]
    W2r = W2[routing]  # [A, H, NACT]
    bias1e = b1[routing] + W1r[np.arange(A), H + np.arange(A), :]  # [A, H]
    b2e = b2[routing]  # [A, NACT]
    W1h = W1r[:, :H, :]  # [A, H, H]

    x_bf = x_in.astype(BF)  # [B, A, H]

    # [A, H, H] -> [A, 128, KC, MC, 128]: [a, kp, k, m, mc] = W[a, k*128+kp, m*128+mc]
    w1_all = np.ascontiguousarray(
        W1h.astype(BF).reshape(A, KC, 128, MC, 128).transpose(0, 2, 1, 3, 4)
    )
    w2_all = np.ascontiguousarray(
        W2r.astype(BF).reshape(A, KC, 128, NACT).transpose(0, 2, 1, 3)
    )  # [A, 128, KC, NACT]

    bs_all = np.zeros((A, 128, MC + 1), dtype=np.float32)
    bs_all[:, :, :MC] = bias1e.reshape(A, MC, 128).transpose(0, 2, 1)
    bs_all[:, :NACT, MC] = b2e

    per_core = _unit_tables()
    in_maps = []
    for c in range(N_CORES):
        units, agents = per_core[c]
        xt = np.empty((N_UNITS, 128, KC, BT), dtype=BF)
        for u, (a, b0) in enumerate(units):
            # [kp, k, b] = x[b0+b, a, k*128+kp]
            xt[u] = x_bf[b0 : b0 + BT, a, :].T.reshape(KC, 128, BT).transpose(1, 0, 2)
        in_maps.append(
            {
                "xt": xt,
                "w1": np.ascontiguousarray(w1_all[agents]),
                "w2": w2_all[agents],
                "bs": bs_all[agents],
            }
        )
    return in_maps, per_core


def _install_ntff_hook():
    import types

    try:
        from antenv.axon_hooks import get_axon_ntff_profile_hook  # noqa: F401

        return
    except ImportError:
        pass
    try:
        import antenv
        from trn_agent_boot.trn_boot import _ntff_profile_via_ctypes

        hook = _ntff_profile_via_ctypes("/opt/axon/libaxon_pjrt.so")
        mod = types.ModuleType("antenv.axon_hooks")
        mod.get_axon_ntff_profile_hook = lambda: hook
        mod.set_axon_ntff_profile_hook = lambda h: None
        sys.modules["antenv.axon_hooks"] = mod
        antenv.axon_hooks = mod
    except Exception:
        pass


def kernel(x_in, W1, b1, W2, b2, routing):
    from concourse.bass_utils import run_bass_kernel_spmd

    trace = bool(os.environ.get("TRN_KERNEL_TRACE"))
    if trace:
        _install_ntff_hook()

    if "nc" not in _CACHE:
        _CACHE["nc"] = _build_nc()
    nc = _CACHE["nc"]

    in_maps, per_core = _prep_inputs(x_in, W1, b1, W2, b2, routing)

    kwargs = {}
    if trace:
        kwargs = dict(trace=True, tmpdir=os.environ.get("TRN_KERNEL_TRACE_DIR") or None)
    res = run_bass_kernel_spmd(nc, in_maps, core_ids=list(range(N_CORES)), **kwargs)

    LAST_RUN_INFO.clear()
    LAST_RUN_INFO["exec_time_ns"] = res.exec_time_ns
    LAST_RUN_INFO["results"] = res

    out_full = np.empty((B, A, NACT), dtype=np.float32)
    for c in range(N_CORES):
        units, agents = per_core[c]
        oc = res.results[c]["out"]  # [N_AG, NACT, 4*BT]
        for ai, a in enumerate(agents):
            if ai == FULL_PER_CORE:
                b0 = units[-1][1]
                out_full[b0 : b0 + BT, a, :] = oc[ai, :, :BT].T
            else:
                out_full[:, a, :] = oc[ai].T
    return out_full


# revision 17
# speedup vs baseline: 1.0391x; 1.0036x over previous
"""TRN2 Bass kernel for nn_DivTree (moe_routing): per-agent 2-layer MLP.

Math (per batch row b, agent a, with r = routing[a]):
    x0   = concat(x_in[b, a], onehot(a))                  # [H + A]
    h    = relu(x0 @ W1[r] + b1[r])                       # [H]
    out  = h @ W2[r] + b2[r]                              # [NACT]

Host-side simplifications baked in before the device kernel runs:
  - The onehot half of x0 @ W1[r] selects row H+a of W1[r]; folded into an
    effective bias: bias1e[a] = b1[r] + W1[r, H+a, :].
  - Expert weights gathered by routing on the host (pure indexing).
  - x and W cast to bf16 (measured end-to-end err ~2.8e-3 rel); fp32 PSUM.
    bf16 matmuls stream 1 row/cycle like fp32r but their 32KB weight loads
    hide fully under the 213ns stream (fp32r's 64KB loads don't), and DMA
    traffic halves. fp8 DoubleRow was measured at 2x MACs/instr wall-equal,
    which loses once the >=2 correction terms needed for accuracy are paid.

Sharding: expert-parallel over agents. 48 agents assigned whole to cores
(6 each); agents 48/49 split into 4 batch-quarters (cores 0-3 / 4-7), so
all 8 cores run an identical program over 25 (agent, batch-512) units.

Device kernel per unit: 16 bf16 matmuls (L1) + 4 bf16 matmuls (L2, flushed
one unit late to keep the PE streaming). Relu+bias+bf16-cast runs on the
Vector engine (tensor_scalar add+max), keeping Scalar free for weight DMA;
xt DMA on Sync, output DMA on GpSimd.
"""

import os
import sys

import numpy as np

sys.path.insert(0, "/opt/trn_rl_repo")

B, A, H, NACT = 2048, 50, 512, 64
N_CORES = 8
BT = 512  # batch tile (rows per work unit)
FULL_PER_CORE = 6  # whole agents per core
N_UNITS = FULL_PER_CORE * 4 + 1  # 25 work units per core
N_AG = FULL_PER_CORE + 1  # weight slots per core (6 full + 1 split)
KC = H // 128  # 4 contraction k-tiles
MC = H // 128  # 4 output-hidden chunks

LAST_RUN_INFO = {}

_CACHE = {}


def _unit_tables():
    """Per-core unit -> (agent, b0) and weight-slot tables."""
    per_core = []
    for c in range(N_CORES):
        full = list(range(c * FULL_PER_CORE, (c + 1) * FULL_PER_CORE))
        split_agent = 48 + (c // 4)
        quarter = c % 4
        units = [(a, j * BT) for a in full for j in range(4)]
        units.append((split_agent, quarter * BT))
        agents = full + [split_agent]
        per_core.append((units, agents))
    return per_core


def _build_nc():
    import concourse.bacc as bacc
    import concourse.mybir as mybir
    import concourse.tile as tile

    F32 = mybir.dt.float32
    BF16 = mybir.dt.bfloat16
    ADD = mybir.AluOpType.add
    MAX = mybir.AluOpType.max
    Relu = mybir.ActivationFunctionType.Relu

    nc = bacc.Bacc(None)
    # x: [unit][partition][ktile][batch]
    xt_d = nc.declare_dram_parameter("xt", [N_UNITS, 128, KC, BT], BF16, isOutput=False)
    # W1: [agent][partition][ktile][m][col]
    w1_d = nc.declare_dram_parameter(
        "w1", [N_AG, 128, KC, MC, 128], BF16, isOutput=False
    )
    w2_d = nc.declare_dram_parameter("w2", [N_AG, 128, KC, NACT], BF16, isOutput=False)
    # bias: cols 0..MC-1 = bias1e (m-chunks), col MC = b2
    bs_d = nc.declare_dram_parameter("bs", [N_AG, 128, MC + 1], F32, isOutput=False)
    out_d = nc.declare_dram_parameter("out", [N_AG, NACT, 4 * BT], F32, isOutput=True)

    with tile.TileContext(nc) as tc:
        with (
            tc.tile_pool(name="xtp", bufs=8) as xtp,
            tc.tile_pool(name="w1p", bufs=4) as w1p,
            tc.tile_pool(name="w2p", bufs=4) as w2p,
            tc.tile_pool(name="bsp", bufs=4) as bsp,
            tc.tile_pool(name="htp", bufs=12) as htp,
            tc.tile_pool(name="obp", bufs=3) as obp,
            tc.tile_pool(name="ps1p", bufs=6, space="PSUM") as ps1p,
            tc.tile_pool(name="ps2p", bufs=2, space="PSUM") as ps2p,
        ):
            w1ts, w2ts, bsts = {}, {}, {}

            def emit_wslab(ai, split_first=False):
                w1t = w1p.tile([128, KC, MC, 128], BF16, tag="w1", name=f"w1_{ai}")
                if split_first:
                    # stream k-by-k so unit 0's k-outer matmuls start early
                    for k in range(KC):
                        nc.scalar.dma_start(out=w1t[:, k], in_=w1_d[ai][:, k])
                else:
                    nc.scalar.dma_start(out=w1t, in_=w1_d[ai])
                w2t = w2p.tile([128, KC, NACT], BF16, tag="w2", name=f"w2_{ai}")
                nc.scalar.dma_start(out=w2t, in_=w2_d[ai])
                bst = bsp.tile([128, MC + 1], F32, tag="bs", name=f"bs_{ai}")
                nc.scalar.dma_start(out=bst, in_=bs_d[ai])
                w1ts[ai], w2ts[ai], bsts[ai] = w1t, w2t, bst

            def flush(p):
                hts, ai, u, j = p
                ps2 = ps2p.tile([128, BT], F32, tag="ps2", name=f"ps2_{u}")
                w2t = w2ts[ai]
                for k in range(KC):
                    nc.tensor.matmul(
                        ps2[0:NACT, :],
                        lhsT=w2t[:, k, :],
                        rhs=hts[k],
                        start=(k == 0),
                        stop=(k == KC - 1),
                    )
                ob = obp.tile([NACT, BT], F32, tag="ob", name=f"ob_{u}")
                nc.vector.tensor_scalar_add(
                    out=ob, in0=ps2[0:NACT, :], scalar1=bsts[ai][0:NACT, MC : MC + 1]
                )
                nc.gpsimd.dma_start(out=out_d[ai][:, j * BT : (j + 1) * BT], in_=ob)

            emit_wslab(0, split_first=True)

            # Warm the PE (clock ramp) with dummy bf16 matmuls while the
            # first xt/w1 DMAs stream in.
            warm = htp.tile([128, BT + NACT], BF16, tag="warm", name="warm", bufs=1)
            nc.gpsimd.memset(warm, 0.0)
            wps = ps2p.tile([128, BT], F32, tag="ps2", name="warm_ps")
            NWARM = 4
            for r in range(NWARM):
                nc.tensor.matmul(
                    wps[0:NACT, : BT // 2],
                    lhsT=warm[:, BT : BT + NACT],
                    rhs=warm[:, : BT // 2],
                    start=(r == 0),
                    stop=(r == NWARM - 1),
                )

            pending = None
            for u in range(N_UNITS):
                ai = u // 4 if u < FULL_PER_CORE * 4 else FULL_PER_CORE
                j = (u % 4) if ai != FULL_PER_CORE else 0

                xt = xtp.tile([128, KC, BT], BF16, tag="xt", name=f"xt_{u}")
                if u == 0:
                    for k in range(KC):
                        nc.sync.dma_start(out=xt[:, k], in_=xt_d[u][:, k])
                else:
                    nc.sync.dma_start(out=xt, in_=xt_d[u])
                if u % 4 == 0 and u > 0 and u // 4 + 1 <= FULL_PER_CORE:
                    emit_wslab(u // 4 + 1)  # one-agent prefetch lead
                if u == 1:
                    emit_wslab(1)

                w1t = w1ts[ai]
                ps1s = [
                    ps1p.tile([128, BT], F32, tag="ps1", name=f"ps1_{u}_{m}")
                    for m in range(MC)
                ]
                # Unit 0 runs k-outer so each arriving xt/w1 k-chunk unlocks
                # 4 matmuls; later units run m-outer so each psum completes
                # early and its relu overlaps the next m.
                order = (
                    [(m, k) for k in range(KC) for m in range(MC)]
                    if u == 0
                    else [(m, k) for m in range(MC) for k in range(KC)]
                )
                hts = [None] * MC
                for m, k in order:
                    nc.tensor.matmul(
                        ps1s[m],
                        lhsT=w1t[:, k, m, :],
                        rhs=xt[:, k, :],
                        start=(k == 0),
                        stop=(k == KC - 1),
                    )
                    if k == KC - 1:
                        ht = htp.tile([128, BT], BF16, tag="ht", name=f"ht_{u}_{m}")
                        # alternate relu between ACT and DVE so neither
                        # engine's psum drain gates the PE's psum reuse
                        if m % 2 == 0:
                            nc.scalar.activation(
                                out=ht,
                                in_=ps1s[m],
                                func=Relu,
                                bias=bsts[ai][:, m : m + 1],
                            )
                        else:
                            nc.vector.tensor_scalar(
                                out=ht,
                                in0=ps1s[m],
                                scalar1=bsts[ai][:, m : m + 1],
                                scalar2=0.0,
                                op0=ADD,
                                op1=MAX,
                            )
                        hts[m] = ht

                if pending is not None:
                    flush(pending)
                pending = (hts, ai, u, j)
            flush(pending)

    nc.finalize()
    return nc


def _prep_inputs(x_in, W1, b1, W2, b2, routing):
    """Host-side: routing gather, onehot fold, bf16 cast, per-core tiling."""
    import ml_dtypes

    BF = ml_dtypes.bfloat16

    x_in = np.ascontiguousarray(x_in, dtype=np.float32)
    W1 = np.asarray(W1, dtype=np.float32)
    b1 = np.asarray(b1, dtype=np.float32)
    W2 = np.asarray(W2, dtype=np.float32)
    b2 = np.asarray(b2, dtype=np.float32)
    routing = np.asarray(routing)

    W1r = W1[routing]  # [A, H+A, H]
    W2r = W2[routing]  # [A, H, NACT]
    bias1e = b1[routing] + W1r[np.arange(A), H + np.arange(A), :]  # [A, H]
    b2e = b2[routing]  # [A, NACT]
    W1h = W1r[:, :H, :]  # [A, H, H]

    x_bf = x_in.astype(BF)  # [B, A, H]

    # [A, H, H] -> [A, 128, KC, MC, 128]: [a, kp, k, m, mc] = W[a, k*128+kp, m*128+mc]
    w1_all = np.ascontiguousarray(
        W1h.astype(BF).reshape(A, KC, 128, MC, 128).transpose(0, 2, 1, 3, 4)
    )
    w2_all = np.ascontiguousarray(
        W2r.astype(BF).reshape(A, KC, 128, NACT).transpose(0, 2, 1, 3)
    )  # [A, 128, KC, NACT]

    bs_all = np.zeros((A, 128, MC + 1), dtype=np.float32)
    bs_all[:, :, :MC] = bias1e.reshape(A, MC, 128).transpose(0, 2, 1)
    bs_all[:, :NACT, MC] = b2e

    per_core = _unit_tables()
    in_maps = []
    for c in range(N_CORES):
        units, agents = per_core[c]
        xt = np.empty((N_UNITS, 128, KC, BT), dtype=BF)
        for u, (a, b0) in enumerate(units):
            # [kp, k, b] = x[b0+b, a, k*128+kp]
            xt[u] = x_bf[b0 : b0 + BT, a, :].T.reshape(KC, 128, BT).transpose(1, 0, 2)
        in_maps.append(
            {
                "xt": xt,
                "w1": np.ascontiguousarray(w1_all[agents]),
                "w2": w2_all[agents],
                "bs": bs_all[agents],
            }
        )
    return in_maps, per_core


def _install_ntff_hook():
    import types

    try:
        from antenv.axon_hooks import get_axon_ntff_profile_hook  # noqa: F401

        return
    except ImportError:
        pass
    try:
        import antenv
        from trn_agent_boot.trn_boot import _ntff_profile_via_ctypes

        hook = _ntff_profile_via_ctypes("/opt/axon/libaxon_pjrt.so")
        mod = types.ModuleType("antenv.axon_hooks")
        mod.get_axon_ntff_profile_hook = lambda: hook
        mod.set_axon_ntff_profile_hook = lambda h: None
        sys.modules["antenv.axon_hooks"] = mod
        antenv.axon_hooks = mod
    except Exception:
        pass


def kernel(x_in, W1, b1, W2, b2, routing):
    from concourse.bass_utils import run_bass_kernel_spmd

    trace = bool(os.environ.get("TRN_KERNEL_TRACE"))
    if trace:
        _install_ntff_hook()

    if "nc" not in _CACHE:
        _CACHE["nc"] = _build_nc()
    nc = _CACHE["nc"]

    in_maps, per_core = _prep_inputs(x_in, W1, b1, W2, b2, routing)

    kwargs = {}
    if trace:
        kwargs = dict(trace=True, tmpdir=os.environ.get("TRN_KERNEL_TRACE_DIR") or None)
    res = run_bass_kernel_spmd(nc, in_maps, core_ids=list(range(N_CORES)), **kwargs)

    LAST_RUN_INFO.clear()
    LAST_RUN_INFO["exec_time_ns"] = res.exec_time_ns
    LAST_RUN_INFO["results"] = res

    out_full = np.empty((B, A, NACT), dtype=np.float32)
    for c in range(N_CORES):
        units, agents = per_core[c]
        oc = res.results[c]["out"]  # [N_AG, NACT, 4*BT]
        for ai, a in enumerate(agents):
            if ai == FULL_PER_CORE:
                b0 = units[-1][1]
                out_full[b0 : b0 + BT, a, :] = oc[ai, :, :BT].T
            else:
                out_full[:, a, :] = oc[ai].T
    return out_full
